# revision 13
# baseline (speedup 1.0000x reference)
"""Self-attention layer (softmax(X @ X^T) @ X) on 8 Trainium2 NeuronCores.

Data-parallel over batch: each of the 8 cores computes one batch element's
full attention for X of shape [2048, 512].

Per-core algorithm. Scores are computed TRANSPOSED (T[j, i] = <x_i, x_j>,
key index j on partitions) so the exponentiated tile is already in the
layout the PV matmul needs as its stationary operand. QK matmuls run in
fp8-e4m3 DoubleRow (2 MACs/cell/cycle, K=256 per matmul) — score rounding
cancels exactly through the l-normalization. PV runs in bf16 for output
precision. The softmax stabilizer c[i] = |x_i|^2 is subtracted on the
vector engine from a pre-broadcast [128, S] row; l and 1/l come from an
N=1 matmul sharing the PV matmuls' loaded weights.

  1. Load X (per-tile pipelined): X_bf = bf16(X); PE-transpose X_bf into
     Xt8 (fp8, [d, s] layout).
  2. Per query chunk: sq = Xt8*Xt8 (bf16); c row via ones-matmul over sq;
     broadcast -c into negc_full [128, S] (K=1 matmul + copy).
  3. For each query chunk ic (512 queries), for each key block jt:
       psum = QK fp8 DoubleRow matmuls (2 x K=256)
       psum -= c (vector engine)
       E_T[jt block][:, ic] = exp(psum)   (scalar engine, direct to SBUF)
  4. PV per query block i (software-pipelined one chunk behind QK):
       po = sum_j E_T[:, j-block, i]^T @ X_bf[j]   (bf16)
       l  = same loaded weights @ ones (N=1);  O_i = po / l; DMA out.
"""

import os
import numpy as np

B, S, D = 8, 2048, 512
P = 128
NI = S // P  # 16 row blocks
NK = D // P  # 4 d-tiles
JC = 512     # query column chunk (one psum bank)
NC = S // JC  # 4 chunks
NSUB = JC // P  # 4 i-tiles per chunk

_CACHE = {}


def _build_nc():
    from contextlib import ExitStack

    import concourse.bacc as bacc
    import concourse.mybir as mybir
    import concourse.tile as tile
    from concourse import masks

    f32 = mybir.dt.float32
    bf16 = mybir.dt.bfloat16
    fp8 = mybir.dt.float8e4
    AF = mybir.ActivationFunctionType
    DR = mybir.MatmulPerfMode.DoubleRow

    nc = bacc.Bacc("TRN2", target_bir_lowering=False, debug=False, num_devices=B)
    inp = nc.dram_tensor("inputs", [S, D], f32, kind="ExternalInput").ap()
    out = nc.dram_tensor("out", [S, D], f32, kind="ExternalOutput").ap()

    with tile.TileContext(nc) as tc, ExitStack() as ctx:
        const_pool = ctx.enter_context(tc.tile_pool(name="const", bufs=1))
        persist = ctx.enter_context(tc.tile_pool(name="persist", bufs=1))
        xin_pool = ctx.enter_context(tc.tile_pool(name="xin", bufs=4))
        stat_pool = ctx.enter_context(tc.tile_pool(name="stat", bufs=3))
        osb_pool = ctx.enter_context(tc.tile_pool(name="osb", bufs=3))
        # PSUM budget (8 banks): qk 3 + pt 2 + pv 2 + lc 1
        qk_psum = ctx.enter_context(tc.tile_pool(name="qk_psum", bufs=3, space="PSUM"))
        tr_psum = ctx.enter_context(tc.tile_pool(name="tr_psum", bufs=2, space="PSUM"))
        pv_psum = ctx.enter_context(tc.tile_pool(name="pv_psum", bufs=2, space="PSUM"))
        l_psum = ctx.enter_context(tc.tile_pool(name="l_psum", bufs=1, space="PSUM"))

        ident = const_pool.tile([P, P], bf16, tag="ident", name="ident")
        masks.make_identity(nc, ident[:])
        ones_row = const_pool.tile([1, P], bf16, tag="ones_row", name="ones_row")
        nc.vector.memset(ones_row[:], 1.0)
        ones_col = const_pool.tile([P, 1], bf16, tag="ones_col", name="ones_col")
        nc.vector.memset(ones_col[:], 1.0)

        X_bf = persist.tile([P, NI * D], bf16, tag="xbf", name="xbf")
        Xt8 = persist.tile([P, NK * S], fp8, tag="xt8", name="xt8")
        sq = persist.tile([P, NK * S], bf16, tag="sq", name="sq")
        negc = persist.tile([1, S], bf16, tag="negc", name="negc")
        negc_full = persist.tile([P, S], bf16, tag="negc_full", name="negc_full")
        E_T = persist.tile([P, NI * S], bf16, tag="et", name="et")

        Xt8_3 = Xt8[:].rearrange("p (k s) -> p k s", k=NK)
        Xt8_4 = Xt8[:].rearrange("p (k2 two s) -> p k2 two s", k2=NK // 2, two=2)
        sq3 = sq[:].rearrange("p (k s) -> p k s", k=NK)

        # ---- emit helpers ----
        def emit_load_tile(i):
            dcols = slice(i * D, (i + 1) * D)
            xf = xin_pool.tile([P, D], f32, tag="xf", name=f"xf{i}")
            nc.sync.dma_start(xf[:], inp[i * P : (i + 1) * P, :])
            if i % 2 == 0:
                nc.scalar.copy(X_bf[:, dcols], xf[:])
            else:
                nc.vector.tensor_copy(X_bf[:, dcols], xf[:])
            pt = tr_psum.tile([P, NK, P], bf16, tag="pt", name=f"ptx{i}")
            for k in range(NK):
                nc.tensor.matmul(
                    pt[:, k],
                    lhsT=X_bf[:, i * D + k * P : i * D + (k + 1) * P],
                    rhs=ident[:],
                    is_transpose=True,
                    skip_group_check=True,
                )
            nc.vector.tensor_copy(Xt8_3[:, :, i * P : (i + 1) * P], pt[:])

        def emit_sq_negc(ic):
            # negc_full[p, s] = -sum_d X[s, d]^2, for chunk ic's columns
            ccols = slice(ic * JC, (ic + 1) * JC)
            for k in range(NK):
                nc.vector.tensor_mul(
                    sq3[:, k, ccols], Xt8_3[:, k, ccols], Xt8_3[:, k, ccols]
                )
            pc = l_psum.tile([1, JC], f32, tag="lc", name=f"c{ic}")
            for k in range(NK):
                nc.tensor.matmul(
                    pc[:],
                    lhsT=ones_col[:],
                    rhs=sq3[:, k, ccols],
                    start=(k == 0),
                    stop=(k == NK - 1),
                )
            nc.scalar.mul(negc[:, ccols], pc[:], -1.0)
            pb = tr_psum.tile([P, JC], f32, tag="pt", name=f"pb{ic}")
            nc.tensor.matmul(pb[:], lhsT=ones_row[:], rhs=negc[:, ccols])
            nc.vector.tensor_copy(negc_full[:, ccols], pb[:])

        def emit_qk_tile(ic, jt):
            ccols = slice(ic * JC, (ic + 1) * JC)
            ps = qk_psum.tile([P, JC], f32, tag="qk", name=f"qk{ic}_{jt}")
            for k2 in range(NK // 2):
                nc.tensor.matmul(
                    ps[:],
                    lhsT=Xt8_4[:, k2, :, jt * P : (jt + 1) * P],
                    rhs=Xt8_4[:, k2, :, ccols],
                    perf_mode=DR,
                    start=(k2 == 0),
                    stop=(k2 == NK // 2 - 1),
                )
            nc.vector.tensor_add(ps[:], ps[:], negc_full[:, ccols])
            nc.scalar.activation(
                E_T[:, jt * S + ic * JC : jt * S + (ic + 1) * JC],
                ps[:],
                AF.Exp,
            )

        def emit_qk(ic):
            for jt in range(NI):
                emit_qk_tile(ic, jt)

        # ---- startup: stream tiles in, interleaving QK chunk 0 tiles so the
        # tensor engine starts scoring as soon as each key block's transpose
        # lands (instead of idling behind all 64 transposes) ----
        for i in range(NI):
            emit_load_tile(i)
            if i == NSUB - 1:
                emit_sq_negc(0)
                for jt in range(NSUB):
                    emit_qk_tile(0, jt)
            elif i >= NSUB:
                emit_qk_tile(0, i)
                if i % NSUB == NSUB - 1:
                    emit_sq_negc(i // NSUB)

        def emit_pv(i):
            po = pv_psum.tile([P, D], f32, tag="pv", name=f"pv{i}")
            pl = l_psum.tile([P, 1], f32, tag="lc", name=f"l{i}")
            for j in range(NI):
                lhsT = E_T[:, j * S + i * P : j * S + (i + 1) * P]
                nc.tensor.matmul(
                    po[:],
                    lhsT=lhsT,
                    rhs=X_bf[:, j * D : (j + 1) * D],
                    start=(j == 0),
                    stop=(j == NI - 1),
                )
                nc.tensor.matmul(
                    pl[:],
                    lhsT=lhsT,
                    rhs=ones_col[:],
                    start=(j == 0),
                    stop=(j == NI - 1),
                )
            linv = stat_pool.tile([P, 1], f32, tag="linv", name=f"linv{i}")
            nc.vector.reciprocal(linv[:], pl[:])
            osb = osb_pool.tile([P, D], f32, tag="osb", name=f"osb{i}")
            nc.vector.tensor_scalar_mul(osb[:], po[:], linv[:])
            nc.sync.dma_start(out[i * P : (i + 1) * P, :], osb[:])

        # chunk 0's QK was emitted during the load stream above
        for ic in range(1, NC + 1):
            if ic < NC:
                emit_qk(ic)
            for ii in range(NSUB):
                emit_pv((ic - 1) * NSUB + ii)

    nc.compile()
    return nc


def _maybe_install_trace_hook():
    """Install the NTFF profile hook (test/profiling only; optional)."""
    import sys
    import types

    try:
        from antenv.axon_hooks import get_axon_ntff_profile_hook  # noqa: F401

        return  # already available
    except ImportError:
        pass
    try:
        mod = types.ModuleType("antenv.axon_hooks")
        _hook = [None]
        mod.set_axon_ntff_profile_hook = lambda h: _hook.__setitem__(0, h)
        mod.get_axon_ntff_profile_hook = lambda: _hook[0]
        sys.modules["antenv.axon_hooks"] = mod
        import antenv

        antenv.axon_hooks = mod
        from trn_agent_boot.trn_boot import _ntff_profile_via_ctypes

        mod.set_axon_ntff_profile_hook(
            _ntff_profile_via_ctypes("/opt/axon/libaxon_pjrt.so")
        )
    except Exception:
        pass


def kernel(inputs: np.ndarray) -> np.ndarray:
    from concourse.bass_utils import run_bass_kernel_spmd

    x = np.ascontiguousarray(np.asarray(inputs, dtype=np.float32))
    assert x.shape == (B, S, D), f"unexpected input shape {x.shape}"

    if "nc" not in _CACHE:
        _CACHE["nc"] = _build_nc()
    nc = _CACHE["nc"]

    trace = bool(int(os.environ.get("ATT_KERNEL_TRACE", "0")))
    if trace:
        _maybe_install_trace_hook()

    in_maps = [{"inputs": x[b]} for b in range(B)]
    res = run_bass_kernel_spmd(nc, in_maps, core_ids=list(range(B)), trace=trace)
    kernel.last_exec_time_ns = res.exec_time_ns
    return np.stack([res.results[b]["out"] for b in range(B)], axis=0)


kernel.last_exec_time_ns = None
